# revision 14
# baseline (speedup 1.0000x reference)
"""Trainium2 Bass kernel for nn_Mix_82360292868539.

reference math:
    inner = x @ y.T                                   # [8192, 8192] fp32
    pdist = sx[:,None] + sy[None,:] - 2*inner
    sigma = median(pdist) / (2*log(8193))
    kxy   = exp(-pdist/sigma/2) + 0.1*(inner + 0)**2

Design (vs the 185us baseline, which was ACT-bound: Square+Exp both on the
scalar engine = 16M 1x-elems/core = ~128us busy):

  * sigma from a host-side subsample median (x[::16] vs y[::16], 262144
    samples, rel SE ~4e-4 on the median). The exp term is only ~1.4e-4 of
    the output L2 norm, so sigma precision is irrelevant at the 2e-2 gate.
    This deletes the entire device pass 1 (~35us).
  * fp16 output: halves the dominant out-DMA (32MB -> 16MB per core).
  * forked PSUM chains, one [128,1024] tile pair per group:
      MM_P = C1*x.yh                     (pure inner; ACT squares it)
      MM_E = C1*x~.yh - C1*(sx_i+sy_j)/2 (= -C1*pdist/2; the full aug rides
             in two sacrificial K-rows: lhsT [uhi; ulo[:62]; ax; 1],
             rhs [yh; yh[:62]; 1; cy], so x~ drops x_lo features 62-63,
             err ~4e-4 on the exp arg)
    No square->aug->exp WAR chain; each PSUM bank has one writer then one
    reader, so 2 bufs per pool pipeline cleanly.
  * exp WITHOUT the scalar engine: Schraudolph bit-trick on DVE. One
    tensor_scalar from PSUM computes i = W_E*(A16/(C1*sigma)) + B16 with
    uint16 output; the uint16 bit pattern IS fp16(exp(z)) (exponent-field
    linear interpolation, max rel err ~3%, invisible at E's 1.4e-4 L2
    share). Negative i (exp underflow, z < -10.4) saturates to 0 on the
    unsigned convert = exactly the clamp needed. The add then reads the
    tile bitcast as fp16.
  * adds in fp16 (DVE 2x_1P mode), a slice of groups routed to GpSimd, and
    optionally a slice of exps routed to ACT (exact Exp), to balance the
    three engines. Knobs: exp_act_mod, gp_mod.

Error budget: P misses x.ylo (~2.3e-3 L2, dominant), fp16 out ~2.4e-4,
Schraudolph ~4e-6, sigma ~1e-6. Total ~2.4e-3 vs the 2e-2 gate.
"""

import math
import numpy as np

import jax
from jax.sharding import Mesh, PartitionSpec, NamedSharding
from jax.experimental.shard_map import shard_map

import bass_rust
import ml_dtypes
import concourse.bass as bass
import concourse.mybir as mybir
from concourse.tile import TileContext

BF16 = ml_dtypes.bfloat16

N, M, D = 8192, 8192, 64
R_POLY = 0.1
N_CORES = 8
ROWS = N // N_CORES          # 1024 rows per core
C1 = math.sqrt(R_POLY)       # sqrt(0.1) folded into x side of the matmuls

F_TILE = 1024                # output-tile free dim (2 PSUM banks)
RB = ROWS // 128             # row blocks per core (8)
CT = M // F_TILE             # column tiles (8)

# fp16 Schraudolph constants: bitcast_u16(round(A16*z + B16)) ~= fp16(exp(z))
A16 = 1024.0 / math.log(2.0)          # 2^10 * log2(e)
B16 = (15.0 - 0.0434609) * 1024.0 + 0.5   # +0.5: correct under truncation


def _split_multiwait_ctrl(nc, maxw=1):
    """This container's walrus build only accepts one sem-wait command per
    instruction. Split any multi-wait instruction into a chain of
    single-wait NoOps (same engine, program order preserved) followed by
    the original instruction carrying the final wait."""
    for f in nc.m.functions:
        for bb in f.blocks:
            new = []
            for inst in bb.instructions:
                si = inst.sync_info
                ws = list(si.on_wait) if si and si.on_wait else []
                if len(ws) > maxw and inst.engine is not None:
                    for i, w in enumerate(ws[:-maxw]):
                        d = mybir.InstNoOp(name=f"{inst.name}-sw{i}", ins=[], outs=[])
                        d.engine = inst.engine
                        d.sync_info = bass_rust.SyncInfo(on_wait=[w], on_update=[])
                        new.append(d)
                    si.on_wait = ws[-maxw:]
                new.append(inst)
            bb.instructions = new


def build_pass2(exp_act_mod=3, gp_mod=2, sbuf_bufs=4, repeat=1, timing=False,
                no_dma=False, pair_dma=False, add_pair=True):
    """exp_act_mod: every exp_act_mod-th group's exp runs exactly on ACT
    instead of the DVE bit-trick (0 = all DVE). gp_mod: every gp_mod-th
    group's add runs on GpSimd instead of DVE (0 = all DVE)."""
    nc = bass.Bass("TRN2", target_bir_lowering=False, num_devices=N_CORES)
    u2T = nc.dram_tensor("u2T", [128, ROWS], mybir.dt.bfloat16, kind="ExternalInput")
    uET = nc.dram_tensor("uET", [128, ROWS], mybir.dt.bfloat16, kind="ExternalInput")
    yPT = nc.dram_tensor("yPT", [128, M], mybir.dt.bfloat16, kind="ExternalInput")
    yET = nc.dram_tensor("yET", [128, M], mybir.dt.bfloat16, kind="ExternalInput")
    sAin = nc.dram_tensor("sA", [128, 1], mybir.dt.float32, kind="ExternalInput")
    c2in = nc.dram_tensor("c2", [128, 1], mybir.dt.float32, kind="ExternalInput")
    if timing:
        out = nc.dram_tensor("scratch", [ROWS, M], mybir.dt.float16, kind="Internal")
        tok = nc.dram_tensor("tok", [128, 8], mybir.dt.float16, kind="ExternalOutput")
    else:
        out = nc.dram_tensor("out", [ROWS, M], mybir.dt.float16, kind="ExternalOutput")

    with TileContext(nc) as tc:
        with tc.tile_pool(name="w", bufs=1) as wpool, \
             tc.tile_pool(name="psP", bufs=2, space="PSUM") as ppsum, \
             tc.tile_pool(name="psE", bufs=2, space="PSUM") as epsum, \
             tc.tile_pool(name="pb", bufs=sbuf_bufs) as ppool, \
             tc.tile_pool(name="eb", bufs=sbuf_bufs) as epool, \
             tc.tile_pool(name="ob", bufs=sbuf_bufs) as opool:
            u2 = wpool.tile([128, ROWS], mybir.dt.bfloat16)
            nc.sync.dma_start(out=u2, in_=u2T[:, :])
            uE = wpool.tile([128, ROWS], mybir.dt.bfloat16)
            nc.sync.dma_start(out=uE, in_=uET[:, :])
            sA = wpool.tile([128, 1], mybir.dt.float32)
            nc.sync.dma_start(out=sA, in_=sAin[:, :])
            c2 = wpool.tile([128, 1], mybir.dt.float32)
            nc.sync.dma_start(out=c2, in_=c2in[:, :])
            yP_ch, yE_ch = [], []
            for j in range(CT):
                csl = slice(j * F_TILE, (j + 1) * F_TILE)
                tp = wpool.tile([128, F_TILE], mybir.dt.bfloat16, tag=f"yp{j}")
                nc.sync.dma_start(out=tp, in_=yPT[:, csl])
                yP_ch.append(tp)
                te = wpool.tile([128, F_TILE], mybir.dt.bfloat16, tag=f"ye{j}")
                nc.sync.dma_start(out=te, in_=yET[:, csl])
                yE_ch.append(te)

            g = 0
            pending = None  # (pt, et_fp16_view, rsl, jg) awaiting add + dma
            pair_state = {}  # jg parity -> shared double-width ot tile

            def flush(pend, g_):
                pt_, etv_, rsl_, jg_ = pend
                if add_pair:
                    dsl = slice((jg_ - 1) * F_TILE, (jg_ + 1) * F_TILE)
                    ot = opool.tile([128, 2 * F_TILE], mybir.dt.float16,
                                    tag="otw")
                    if gp_mod and (g_ // 2) % gp_mod == 0:
                        nc.gpsimd.tensor_tensor(ot, pt_, etv_,
                                                mybir.AluOpType.add)
                    else:
                        nc.vector.tensor_tensor(ot, pt_, etv_,
                                                mybir.AluOpType.add)
                    if not no_dma:
                        nc.sync.dma_start(out=out[rsl_, dsl], in_=ot)
                    return ot
                osl_ = slice(jg_ * F_TILE, (jg_ + 1) * F_TILE)
                if pair_dma:
                    # two adjacent column groups share one [128, 2*F_TILE]
                    # tile so the out-DMA moves 4KB per partition line
                    if jg_ % 2 == 0:
                        ot_new = opool.tile(
                            [128, 2 * F_TILE], mybir.dt.float16, tag="otp")
                        pair_state["ot"] = ot_new
                    ot2 = pair_state["ot"]
                    half = ot2[:, (jg_ % 2) * F_TILE:(jg_ % 2 + 1) * F_TILE]
                    if gp_mod and g_ % gp_mod == 0:
                        nc.gpsimd.tensor_tensor(half, pt_, etv_,
                                                mybir.AluOpType.add)
                    else:
                        nc.vector.tensor_tensor(half, pt_, etv_,
                                                mybir.AluOpType.add)
                    if jg_ % 2 == 1 and not no_dma:
                        dsl = slice((jg_ - 1) * F_TILE, (jg_ + 1) * F_TILE)
                        nc.sync.dma_start(out=out[rsl_, dsl], in_=ot2)
                    return ot2
                ot = opool.tile([128, F_TILE], mybir.dt.float16)
                if gp_mod and g_ % gp_mod == 0:
                    nc.gpsimd.tensor_tensor(ot, pt_, etv_, mybir.AluOpType.add)
                else:
                    nc.vector.tensor_tensor(ot, pt_, etv_, mybir.AluOpType.add)
                if not no_dma:
                    nc.sync.dma_start(out=out[rsl_, osl_], in_=ot)
                return ot

            last_ot = None
            pf = {}  # shared double-width pt/et tiles for add_pair mode
            for _rep in range(repeat):
              for rb in range(RB):
                rsl = slice(rb * 128, (rb + 1) * 128)
                for jg in range(CT):
                    osl = slice(jg * F_TILE, (jg + 1) * F_TILE)
                    psP = ppsum.tile([128, F_TILE], mybir.dt.float32)
                    for h in range(F_TILE // 512):
                        hsl = slice(h * 512, (h + 1) * 512)
                        nc.tensor.matmul(psP[:, hsl], lhsT=u2[:, rsl],
                                         rhs=yP_ch[jg][:, hsl],
                                         start=True, stop=True)
                    psE = epsum.tile([128, F_TILE], mybir.dt.float32)
                    for h in range(F_TILE // 512):
                        hsl = slice(h * 512, (h + 1) * 512)
                        nc.tensor.matmul(psE[:, hsl], lhsT=uE[:, rsl],
                                         rhs=yE_ch[jg][:, hsl],
                                         start=True, stop=True)
                    if add_pair:
                        # square/exp of a jg pair write halves of [128, 2*F]
                        # tiles; one add + one DMA per pair
                        if jg % 2 == 0:
                            pt2 = ppool.tile([128, 2 * F_TILE],
                                             mybir.dt.float16, tag="ptp")
                            et2 = epool.tile([128, 2 * F_TILE],
                                             mybir.dt.uint16, tag="etp")
                            pf = {"pt2": pt2, "et2": et2}
                        hs = slice((jg % 2) * F_TILE, (jg % 2 + 1) * F_TILE)
                        nc.scalar.activation(
                            pf["pt2"][:, hs], psP,
                            mybir.ActivationFunctionType.Square)
                        if exp_act_mod and g % exp_act_mod == 0:
                            nc.scalar.activation(
                                pf["et2"][:, hs].bitcast(mybir.dt.float16),
                                psE, mybir.ActivationFunctionType.Exp,
                                scale=c2[:, :])
                        else:
                            nc.vector.tensor_scalar(
                                pf["et2"][:, hs], psE, sA[:, :], B16,
                                mybir.AluOpType.mult, mybir.AluOpType.add)
                        if jg % 2 == 1:
                            if pending is not None:
                                last_ot = flush(pending, g - 1)
                            pending = (pf["pt2"],
                                       pf["et2"][:, :].bitcast(
                                           mybir.dt.float16),
                                       rsl, jg)
                        g += 1
                        continue
                    # P = W_P^2 = 0.1*(x.yh)^2   (ACT; DVE can't square PSUM)
                    pt = ppool.tile([128, F_TILE], mybir.dt.float16)
                    nc.scalar.activation(
                        pt, psP, mybir.ActivationFunctionType.Square)
                    # E = exp(W_E/(C1*sigma))
                    if exp_act_mod and g % exp_act_mod == 0:
                        et = epool.tile([128, F_TILE], mybir.dt.float16)
                        nc.scalar.activation(
                            et, psE, mybir.ActivationFunctionType.Exp,
                            scale=c2[:, :])
                        etv = et[:, :]
                    else:
                        et = epool.tile([128, F_TILE], mybir.dt.uint16)
                        nc.vector.tensor_scalar(
                            et, psE, sA[:, :], B16,
                            mybir.AluOpType.mult, mybir.AluOpType.add)
                        etv = et[:, :].bitcast(mybir.dt.float16)
                    if pending is not None:
                        last_ot = flush(pending, g - 1)
                    pending = (pt, etv, rsl, jg)
                    g += 1
              if pending is not None:
                  last_ot = flush(pending, g - 1)
                  pending = None
              if timing:
                  nc.sync.dma_start(out=tok[:, :], in_=last_ot[:, 0:8])
    _split_multiwait_ctrl(nc)
    return nc


class BassRunner:
    """Persistent PJRT executor for a Bass program."""

    def __init__(self, nc, n_cores):
        from concourse.bass2jax import (
            _bass_exec_p, install_neuronx_cc_hook, partition_id_tensor)
        install_neuronx_cc_hook()
        self.nc = nc
        self.n_cores = n_cores
        partition_name = (
            nc.partition_id_tensor.name if nc.partition_id_tensor else None)

        in_names, out_names, out_avals = [], [], []
        for alloc in nc.m.functions[0].allocations:
            if not isinstance(alloc, mybir.MemoryLocationSet):
                continue
            name = alloc.memorylocations[0].name
            if alloc.kind == "ExternalInput":
                if name != partition_name:
                    in_names.append(name)
            elif alloc.kind == "ExternalOutput":
                out_names.append(name)
                out_avals.append(jax.core.ShapedArray(
                    tuple(alloc.tensor_shape), mybir.dt.np(alloc.dtype)))
        self.in_names = in_names
        self.out_names = out_names
        self.out_avals = out_avals
        all_in_names = in_names + out_names
        if partition_name is not None:
            all_in_names.append(partition_name)

        def _body(*args):
            operands = list(args)
            if partition_name is not None:
                operands.append(partition_id_tensor())
            return tuple(_bass_exec_p.bind(
                *operands,
                out_avals=tuple(out_avals),
                in_names=tuple(all_in_names),
                out_names=tuple(out_names),
                lowering_input_output_aliases=(),
                sim_require_finite=True,
                sim_require_nnan=True,
                nc=nc,
            ))

        devices = jax.devices()[:n_cores]
        self.mesh = Mesh(np.asarray(devices), ("core",))
        self.sharding = NamedSharding(self.mesh, PartitionSpec("core"))
        n_total = len(in_names) + len(out_names)
        self.jitted = jax.jit(
            shard_map(_body, mesh=self.mesh,
                      in_specs=(PartitionSpec("core"),) * n_total,
                      out_specs=(PartitionSpec("core"),) * len(out_names),
                      check_rep=False),
            keep_unused=True,
        )
        self._zero_dev = None

    def stage_inputs(self, in_maps):
        return [
            jax.device_put(
                np.concatenate([np.asarray(m[name]) for m in in_maps], axis=0),
                self.sharding)
            for name in self.in_names
        ]

    def zero_carriers(self):
        if self._zero_dev is None:
            self._zero_dev = [
                jax.device_put(
                    np.zeros((self.n_cores * av.shape[0], *av.shape[1:]),
                             av.dtype), self.sharding)
                for av in self.out_avals
            ]
        return self._zero_dev

    def execute(self, dev_inputs):
        outs = self.jitted(*dev_inputs, *self.zero_carriers())
        for o in outs:
            o.block_until_ready()
        return outs

    def run(self, in_maps):
        outs = self.execute(self.stage_inputs(in_maps))
        res = []
        for c in range(self.n_cores):
            d = {}
            for i, name in enumerate(self.out_names):
                av = self.out_avals[i]
                d[name] = np.asarray(outs[i]).reshape(
                    self.n_cores, *av.shape)[c]
            res.append(d)
        return res


def _bf16_split(a):
    hi = a.astype(BF16)
    lo = (a - hi.astype(np.float32)).astype(BF16)
    return hi, lo


def _sigma(x, y):
    """Median of pdist over a deterministic 512x512 subsample (262144
    entries; rel SE ~4e-4 on the median, invisible at the 2e-2 gate)."""
    xs, ys = x[::16], y[::16]
    inner = xs @ ys.T
    pd = ((xs * xs).sum(1)[:, None] + (ys * ys).sum(1)[None, :]
          - 2.0 * inner)
    return float(np.median(pd)) / (2.0 * math.log(np.float32(N + 1)))


def make_in_maps(x, y, sigma):
    """Host-side prep of all per-core device inputs."""
    sx = (x * x).sum(1)
    sy = (y * y).sum(1)
    u = (C1 * x).astype(np.float32)
    u_hi, u_lo = _bf16_split(u)
    y_hi = y.astype(BF16)
    ax = (-0.5 * C1 * sx).astype(np.float32).astype(BF16)[:, None]
    cy = (-0.5 * C1 * sy).astype(np.float32).astype(BF16)[:, None]
    ones_n = np.ones((N, 1), dtype=BF16)

    # MM_P stationary: [u_hi; u_lo].T  (K=128)
    u2T_full = np.ascontiguousarray(np.concatenate([u_hi, u_lo], axis=1).T)
    # MM_E stationary: [u_hi; u_lo[:, :62]; ax; ones].T
    uET_full = np.ascontiguousarray(
        np.concatenate([u_hi, u_lo[:, :62], ax, ones_n], axis=1).T)
    # moving operands: [yh; yh] and [yh; yh[:, :62]; ones; cy]
    yPT = np.ascontiguousarray(np.concatenate([y_hi, y_hi], axis=1).T)
    yET = np.ascontiguousarray(
        np.concatenate([y_hi, y_hi[:, :62], ones_n, cy], axis=1).T)
    sA = np.full((128, 1), A16 / (C1 * sigma), dtype=np.float32)
    c2 = np.full((128, 1), 1.0 / (C1 * sigma), dtype=np.float32)

    in_maps = []
    for c in range(N_CORES):
        rsl = slice(c * ROWS, (c + 1) * ROWS)
        in_maps.append({
            "u2T": np.ascontiguousarray(u2T_full[:, rsl]),
            "uET": np.ascontiguousarray(uET_full[:, rsl]),
            "yPT": yPT,
            "yET": yET,
            "sA": sA,
            "c2": c2,
        })
    return in_maps


_CACHE = {}


def _runner():
    if "r2" not in _CACHE:
        _CACHE["r2"] = BassRunner(build_pass2(), N_CORES)
    return _CACHE["r2"]


def kernel(x: np.ndarray, y: np.ndarray) -> np.ndarray:
    x = np.ascontiguousarray(np.asarray(x, dtype=np.float32))
    y = np.ascontiguousarray(np.asarray(y, dtype=np.float32))
    assert x.shape == (N, D) and y.shape == (M, D)

    sigma = _sigma(x, y)
    in_maps = make_in_maps(x, y, sigma)
    try:
        res = _runner().run(in_maps)
    except Exception:
        from concourse.bass_utils import run_bass_kernel_spmd
        res = run_bass_kernel_spmd(
            build_pass2(), in_maps, list(range(N_CORES))).results
    out16 = np.concatenate([res[c]["out"] for c in range(N_CORES)], axis=0)
    return out16.astype(np.float32)


# revision 16
# speedup vs baseline: 1.1609x; 1.1609x over previous
"""Trainium2 Bass kernel for nn_Mix_82360292868539.

reference math:
    inner = x @ y.T                                   # [8192, 8192] fp32
    pdist = sx[:,None] + sy[None,:] - 2*inner
    sigma = median(pdist) / (2*log(8193))
    kxy   = exp(-pdist/sigma/2) + 0.1*(inner + 0)**2

Design (vs the 185us baseline, which was ACT-bound: Square+Exp both on the
scalar engine = 16M 1x-elems/core = ~128us busy):

  * sigma from a host-side subsample median (x[::16] vs y[::16], 262144
    samples, rel SE ~4e-4 on the median). The exp term is only ~1.4e-4 of
    the output L2 norm, so sigma precision is irrelevant at the 2e-2 gate.
    This deletes the entire device pass 1 (~35us).
  * fp16 output: halves the dominant out-DMA (32MB -> 16MB per core).
  * forked PSUM chains, one [128,1024] tile pair per group:
      MM_P = C1*x.yh                     (pure inner; ACT squares it)
      MM_E = C1*x~.yh - C1*(sx_i+sy_j)/2 (= -C1*pdist/2; the full aug rides
             in two sacrificial K-rows: lhsT [uhi; ulo[:62]; ax; 1],
             rhs [yh; yh[:62]; 1; cy], so x~ drops x_lo features 62-63,
             err ~4e-4 on the exp arg)
    No square->aug->exp WAR chain; each PSUM bank has one writer then one
    reader, so 2 bufs per pool pipeline cleanly.
  * exp WITHOUT the scalar engine: Schraudolph bit-trick on DVE. One
    tensor_scalar from PSUM computes i = W_E*(A16/(C1*sigma)) + B16 with
    uint16 output; the uint16 bit pattern IS fp16(exp(z)) (exponent-field
    linear interpolation, max rel err ~3%, invisible at E's 1.4e-4 L2
    share). Negative i (exp underflow, z < -10.4) saturates to 0 on the
    unsigned convert = exactly the clamp needed. The add then reads the
    tile bitcast as fp16.
  * adds in fp16 (DVE 2x_1P mode), a slice of groups routed to GpSimd, and
    optionally a slice of exps routed to ACT (exact Exp), to balance the
    three engines. Knobs: exp_act_mod, gp_mod.

Error budget: P misses x.ylo (~2.3e-3 L2, dominant), fp16 out ~2.4e-4,
Schraudolph ~4e-6, sigma ~1e-6. Total ~2.4e-3 vs the 2e-2 gate.
"""

import math
import numpy as np

import jax
from jax.sharding import Mesh, PartitionSpec, NamedSharding
from jax.experimental.shard_map import shard_map

import bass_rust
import ml_dtypes
import concourse.bass as bass
import concourse.mybir as mybir
from concourse.tile import TileContext

BF16 = ml_dtypes.bfloat16

N, M, D = 8192, 8192, 64
R_POLY = 0.1
N_CORES = 8
ROWS = N // N_CORES          # 1024 rows per core
C1 = math.sqrt(R_POLY)       # sqrt(0.1) folded into x side of the matmuls

F_TILE = 1024                # output-tile free dim (2 PSUM banks)
RB = ROWS // 128             # row blocks per core (8)
CT = M // F_TILE             # column tiles (8)

# fp16 Schraudolph constants: bitcast_u16(round(A16*z + B16)) ~= fp16(exp(z))
A16 = 1024.0 / math.log(2.0)          # 2^10 * log2(e)
B16 = (15.0 - 0.0434609) * 1024.0 + 0.5   # +0.5: correct under truncation


def _split_multiwait_ctrl(nc, maxw=1):
    """This container's walrus build only accepts one sem-wait command per
    instruction. Split any multi-wait instruction into a chain of
    single-wait NoOps (same engine, program order preserved) followed by
    the original instruction carrying the final wait."""
    for f in nc.m.functions:
        for bb in f.blocks:
            new = []
            for inst in bb.instructions:
                si = inst.sync_info
                ws = list(si.on_wait) if si and si.on_wait else []
                if len(ws) > maxw and inst.engine is not None:
                    for i, w in enumerate(ws[:-maxw]):
                        d = mybir.InstNoOp(name=f"{inst.name}-sw{i}", ins=[], outs=[])
                        d.engine = inst.engine
                        d.sync_info = bass_rust.SyncInfo(on_wait=[w], on_update=[])
                        new.append(d)
                    si.on_wait = ws[-maxw:]
                new.append(inst)
            bb.instructions = new


def build_pass2(exp_act_mod=3, gp_mod=2, sbuf_bufs=6, repeat=1, timing=False,
                no_dma=False, pair_dma=False, add_pair=True, mm_pair=False):
    """exp_act_mod: every exp_act_mod-th group's exp runs exactly on ACT
    instead of the DVE bit-trick (0 = all DVE). gp_mod: every gp_mod-th
    group's add runs on GpSimd instead of DVE (0 = all DVE)."""
    nc = bass.Bass("TRN2", target_bir_lowering=False, num_devices=N_CORES)
    u2T = nc.dram_tensor("u2T", [128, ROWS], mybir.dt.bfloat16, kind="ExternalInput")
    uET = nc.dram_tensor("uET", [128, ROWS], mybir.dt.bfloat16, kind="ExternalInput")
    yPT = nc.dram_tensor("yPT", [128, M], mybir.dt.bfloat16, kind="ExternalInput")
    yET = nc.dram_tensor("yET", [128, M], mybir.dt.bfloat16, kind="ExternalInput")
    sAin = nc.dram_tensor("sA", [128, 1], mybir.dt.float32, kind="ExternalInput")
    c2in = nc.dram_tensor("c2", [128, 1], mybir.dt.float32, kind="ExternalInput")
    if timing:
        out = nc.dram_tensor("scratch", [ROWS, M], mybir.dt.float16, kind="Internal")
        tok = nc.dram_tensor("tok", [128, 8], mybir.dt.float16, kind="ExternalOutput")
    else:
        out = nc.dram_tensor("out", [ROWS, M], mybir.dt.float16, kind="ExternalOutput")

    with TileContext(nc) as tc:
        with tc.tile_pool(name="w", bufs=1) as wpool, \
             tc.tile_pool(name="psP", bufs=2, space="PSUM") as ppsum, \
             tc.tile_pool(name="psE", bufs=2, space="PSUM") as epsum, \
             tc.tile_pool(name="pb", bufs=sbuf_bufs) as ppool, \
             tc.tile_pool(name="eb", bufs=sbuf_bufs) as epool, \
             tc.tile_pool(name="ob", bufs=sbuf_bufs) as opool:
            u2 = wpool.tile([128, ROWS], mybir.dt.bfloat16)
            nc.sync.dma_start(out=u2, in_=u2T[:, :])
            uE = wpool.tile([128, ROWS], mybir.dt.bfloat16)
            nc.sync.dma_start(out=uE, in_=uET[:, :])
            sA = wpool.tile([128, 1], mybir.dt.float32)
            nc.sync.dma_start(out=sA, in_=sAin[:, :])
            c2 = wpool.tile([128, 1], mybir.dt.float32)
            nc.sync.dma_start(out=c2, in_=c2in[:, :])
            yP_ch, yE_ch = [], []
            for j in range(CT):
                csl = slice(j * F_TILE, (j + 1) * F_TILE)
                tp = wpool.tile([128, F_TILE], mybir.dt.bfloat16, tag=f"yp{j}")
                nc.sync.dma_start(out=tp, in_=yPT[:, csl])
                yP_ch.append(tp)
                te = wpool.tile([128, F_TILE], mybir.dt.bfloat16, tag=f"ye{j}")
                nc.sync.dma_start(out=te, in_=yET[:, csl])
                yE_ch.append(te)

            g = 0
            pending = None  # (pt, et_fp16_view, rsl, jg) awaiting add + dma
            pair_state = {}  # jg parity -> shared double-width ot tile

            def flush(pend, g_):
                pt_, etv_, rsl_, jg_ = pend
                if add_pair:
                    dsl = slice((jg_ - 1) * F_TILE, (jg_ + 1) * F_TILE)
                    ot = opool.tile([128, 2 * F_TILE], mybir.dt.float16,
                                    tag="otw")
                    if gp_mod and (g_ // 2) % gp_mod == 0:
                        nc.gpsimd.tensor_tensor(ot, pt_, etv_,
                                                mybir.AluOpType.add)
                    else:
                        nc.vector.tensor_tensor(ot, pt_, etv_,
                                                mybir.AluOpType.add)
                    if not no_dma:
                        nc.sync.dma_start(out=out[rsl_, dsl], in_=ot)
                    return ot
                osl_ = slice(jg_ * F_TILE, (jg_ + 1) * F_TILE)
                if pair_dma:
                    # two adjacent column groups share one [128, 2*F_TILE]
                    # tile so the out-DMA moves 4KB per partition line
                    if jg_ % 2 == 0:
                        ot_new = opool.tile(
                            [128, 2 * F_TILE], mybir.dt.float16, tag="otp")
                        pair_state["ot"] = ot_new
                    ot2 = pair_state["ot"]
                    half = ot2[:, (jg_ % 2) * F_TILE:(jg_ % 2 + 1) * F_TILE]
                    if gp_mod and g_ % gp_mod == 0:
                        nc.gpsimd.tensor_tensor(half, pt_, etv_,
                                                mybir.AluOpType.add)
                    else:
                        nc.vector.tensor_tensor(half, pt_, etv_,
                                                mybir.AluOpType.add)
                    if jg_ % 2 == 1 and not no_dma:
                        dsl = slice((jg_ - 1) * F_TILE, (jg_ + 1) * F_TILE)
                        nc.sync.dma_start(out=out[rsl_, dsl], in_=ot2)
                    return ot2
                ot = opool.tile([128, F_TILE], mybir.dt.float16)
                if gp_mod and g_ % gp_mod == 0:
                    nc.gpsimd.tensor_tensor(ot, pt_, etv_, mybir.AluOpType.add)
                else:
                    nc.vector.tensor_tensor(ot, pt_, etv_, mybir.AluOpType.add)
                if not no_dma:
                    nc.sync.dma_start(out=out[rsl_, osl_], in_=ot)
                return ot

            last_ot = None
            pf = {}  # shared double-width pt/et tiles for add_pair mode
            for _rep in range(repeat):
              for rb in range(RB):
                rsl = slice(rb * 128, (rb + 1) * 128)
                mmq = {}  # jg -> (psP, psE) when mm_pair pre-issues them
                for jg in range(CT):
                    osl = slice(jg * F_TILE, (jg + 1) * F_TILE)
                    if mm_pair and jg % 2 == 0:
                        # burst both groups' MM_P under one stationary load,
                        # then both MM_E, halving lhsT switches on the PE
                        ps = {}
                        for j2 in (jg, jg + 1):
                            pj = ppsum.tile([128, F_TILE], mybir.dt.float32,
                                            tag="psPq")
                            for h in range(F_TILE // 512):
                                hsl = slice(h * 512, (h + 1) * 512)
                                nc.tensor.matmul(pj[:, hsl], lhsT=u2[:, rsl],
                                                 rhs=yP_ch[j2][:, hsl],
                                                 start=True, stop=True)
                            ps[j2] = pj
                        for j2 in (jg, jg + 1):
                            ej = epsum.tile([128, F_TILE], mybir.dt.float32,
                                            tag="psEq")
                            for h in range(F_TILE // 512):
                                hsl = slice(h * 512, (h + 1) * 512)
                                nc.tensor.matmul(ej[:, hsl], lhsT=uE[:, rsl],
                                                 rhs=yE_ch[j2][:, hsl],
                                                 start=True, stop=True)
                            mmq[j2] = (ps[j2], ej)
                    if mm_pair:
                        psP, psE = mmq[jg]
                    else:
                        psP = ppsum.tile([128, F_TILE], mybir.dt.float32)
                        for h in range(F_TILE // 512):
                            hsl = slice(h * 512, (h + 1) * 512)
                            nc.tensor.matmul(psP[:, hsl], lhsT=u2[:, rsl],
                                             rhs=yP_ch[jg][:, hsl],
                                             start=True, stop=True)
                        psE = epsum.tile([128, F_TILE], mybir.dt.float32)
                        for h in range(F_TILE // 512):
                            hsl = slice(h * 512, (h + 1) * 512)
                            nc.tensor.matmul(psE[:, hsl], lhsT=uE[:, rsl],
                                             rhs=yE_ch[jg][:, hsl],
                                             start=True, stop=True)
                    if add_pair:
                        # square/exp of a jg pair write halves of [128, 2*F]
                        # tiles; one add + one DMA per pair
                        if jg % 2 == 0:
                            pt2 = ppool.tile([128, 2 * F_TILE],
                                             mybir.dt.float16, tag="ptp")
                            et2 = epool.tile([128, 2 * F_TILE],
                                             mybir.dt.uint16, tag="etp")
                            pf = {"pt2": pt2, "et2": et2}
                        hs = slice((jg % 2) * F_TILE, (jg % 2 + 1) * F_TILE)
                        nc.scalar.activation(
                            pf["pt2"][:, hs], psP,
                            mybir.ActivationFunctionType.Square)
                        if exp_act_mod and g % exp_act_mod == 0:
                            nc.scalar.activation(
                                pf["et2"][:, hs].bitcast(mybir.dt.float16),
                                psE, mybir.ActivationFunctionType.Exp,
                                scale=c2[:, :])
                        else:
                            nc.vector.tensor_scalar(
                                pf["et2"][:, hs], psE, sA[:, :], B16,
                                mybir.AluOpType.mult, mybir.AluOpType.add)
                        if jg % 2 == 1:
                            if pending is not None:
                                last_ot = flush(pending, g - 1)
                            pending = (pf["pt2"],
                                       pf["et2"][:, :].bitcast(
                                           mybir.dt.float16),
                                       rsl, jg)
                        g += 1
                        continue
                    # P = W_P^2 = 0.1*(x.yh)^2   (ACT; DVE can't square PSUM)
                    pt = ppool.tile([128, F_TILE], mybir.dt.float16)
                    nc.scalar.activation(
                        pt, psP, mybir.ActivationFunctionType.Square)
                    # E = exp(W_E/(C1*sigma))
                    if exp_act_mod and g % exp_act_mod == 0:
                        et = epool.tile([128, F_TILE], mybir.dt.float16)
                        nc.scalar.activation(
                            et, psE, mybir.ActivationFunctionType.Exp,
                            scale=c2[:, :])
                        etv = et[:, :]
                    else:
                        et = epool.tile([128, F_TILE], mybir.dt.uint16)
                        nc.vector.tensor_scalar(
                            et, psE, sA[:, :], B16,
                            mybir.AluOpType.mult, mybir.AluOpType.add)
                        etv = et[:, :].bitcast(mybir.dt.float16)
                    if pending is not None:
                        last_ot = flush(pending, g - 1)
                    pending = (pt, etv, rsl, jg)
                    g += 1
              if pending is not None:
                  last_ot = flush(pending, g - 1)
                  pending = None
              if timing:
                  nc.sync.dma_start(out=tok[:, :], in_=last_ot[:, 0:8])
    _split_multiwait_ctrl(nc)
    return nc


class BassRunner:
    """Persistent PJRT executor for a Bass program."""

    def __init__(self, nc, n_cores):
        from concourse.bass2jax import (
            _bass_exec_p, install_neuronx_cc_hook, partition_id_tensor)
        install_neuronx_cc_hook()
        self.nc = nc
        self.n_cores = n_cores
        partition_name = (
            nc.partition_id_tensor.name if nc.partition_id_tensor else None)

        in_names, out_names, out_avals = [], [], []
        for alloc in nc.m.functions[0].allocations:
            if not isinstance(alloc, mybir.MemoryLocationSet):
                continue
            name = alloc.memorylocations[0].name
            if alloc.kind == "ExternalInput":
                if name != partition_name:
                    in_names.append(name)
            elif alloc.kind == "ExternalOutput":
                out_names.append(name)
                out_avals.append(jax.core.ShapedArray(
                    tuple(alloc.tensor_shape), mybir.dt.np(alloc.dtype)))
        self.in_names = in_names
        self.out_names = out_names
        self.out_avals = out_avals
        all_in_names = in_names + out_names
        if partition_name is not None:
            all_in_names.append(partition_name)

        def _body(*args):
            operands = list(args)
            if partition_name is not None:
                operands.append(partition_id_tensor())
            return tuple(_bass_exec_p.bind(
                *operands,
                out_avals=tuple(out_avals),
                in_names=tuple(all_in_names),
                out_names=tuple(out_names),
                lowering_input_output_aliases=(),
                sim_require_finite=True,
                sim_require_nnan=True,
                nc=nc,
            ))

        devices = jax.devices()[:n_cores]
        self.mesh = Mesh(np.asarray(devices), ("core",))
        self.sharding = NamedSharding(self.mesh, PartitionSpec("core"))
        n_total = len(in_names) + len(out_names)
        self.jitted = jax.jit(
            shard_map(_body, mesh=self.mesh,
                      in_specs=(PartitionSpec("core"),) * n_total,
                      out_specs=(PartitionSpec("core"),) * len(out_names),
                      check_rep=False),
            keep_unused=True,
        )
        self._zero_dev = None

    def stage_inputs(self, in_maps):
        return [
            jax.device_put(
                np.concatenate([np.asarray(m[name]) for m in in_maps], axis=0),
                self.sharding)
            for name in self.in_names
        ]

    def zero_carriers(self):
        if self._zero_dev is None:
            self._zero_dev = [
                jax.device_put(
                    np.zeros((self.n_cores * av.shape[0], *av.shape[1:]),
                             av.dtype), self.sharding)
                for av in self.out_avals
            ]
        return self._zero_dev

    def execute(self, dev_inputs):
        outs = self.jitted(*dev_inputs, *self.zero_carriers())
        for o in outs:
            o.block_until_ready()
        return outs

    def run(self, in_maps):
        outs = self.execute(self.stage_inputs(in_maps))
        res = []
        for c in range(self.n_cores):
            d = {}
            for i, name in enumerate(self.out_names):
                av = self.out_avals[i]
                d[name] = np.asarray(outs[i]).reshape(
                    self.n_cores, *av.shape)[c]
            res.append(d)
        return res


def _bf16_split(a):
    hi = a.astype(BF16)
    lo = (a - hi.astype(np.float32)).astype(BF16)
    return hi, lo


def _sigma(x, y):
    """Median of pdist over a deterministic 512x512 subsample (262144
    entries; rel SE ~4e-4 on the median, invisible at the 2e-2 gate)."""
    xs, ys = x[::16], y[::16]
    inner = xs @ ys.T
    pd = ((xs * xs).sum(1)[:, None] + (ys * ys).sum(1)[None, :]
          - 2.0 * inner)
    return float(np.median(pd)) / (2.0 * math.log(np.float32(N + 1)))


def make_in_maps(x, y, sigma):
    """Host-side prep of all per-core device inputs."""
    sx = (x * x).sum(1)
    sy = (y * y).sum(1)
    u = (C1 * x).astype(np.float32)
    u_hi, u_lo = _bf16_split(u)
    y_hi = y.astype(BF16)
    ax = (-0.5 * C1 * sx).astype(np.float32).astype(BF16)[:, None]
    cy = (-0.5 * C1 * sy).astype(np.float32).astype(BF16)[:, None]
    ones_n = np.ones((N, 1), dtype=BF16)

    # MM_P stationary: [u_hi; u_lo].T  (K=128)
    u2T_full = np.ascontiguousarray(np.concatenate([u_hi, u_lo], axis=1).T)
    # MM_E stationary: [u_hi; u_lo[:, :62]; ax; ones].T
    uET_full = np.ascontiguousarray(
        np.concatenate([u_hi, u_lo[:, :62], ax, ones_n], axis=1).T)
    # moving operands: [yh; yh] and [yh; yh[:, :62]; ones; cy]
    yPT = np.ascontiguousarray(np.concatenate([y_hi, y_hi], axis=1).T)
    yET = np.ascontiguousarray(
        np.concatenate([y_hi, y_hi[:, :62], ones_n, cy], axis=1).T)
    sA = np.full((128, 1), A16 / (C1 * sigma), dtype=np.float32)
    c2 = np.full((128, 1), 1.0 / (C1 * sigma), dtype=np.float32)

    in_maps = []
    for c in range(N_CORES):
        rsl = slice(c * ROWS, (c + 1) * ROWS)
        in_maps.append({
            "u2T": np.ascontiguousarray(u2T_full[:, rsl]),
            "uET": np.ascontiguousarray(uET_full[:, rsl]),
            "yPT": yPT,
            "yET": yET,
            "sA": sA,
            "c2": c2,
        })
    return in_maps


_CACHE = {}


def _runner():
    if "r2" not in _CACHE:
        _CACHE["r2"] = BassRunner(build_pass2(), N_CORES)
    return _CACHE["r2"]


def kernel(x: np.ndarray, y: np.ndarray) -> np.ndarray:
    x = np.ascontiguousarray(np.asarray(x, dtype=np.float32))
    y = np.ascontiguousarray(np.asarray(y, dtype=np.float32))
    assert x.shape == (N, D) and y.shape == (M, D)

    sigma = _sigma(x, y)
    in_maps = make_in_maps(x, y, sigma)
    try:
        res = _runner().run(in_maps)
    except Exception:
        from concourse.bass_utils import run_bass_kernel_spmd
        res = run_bass_kernel_spmd(
            build_pass2(), in_maps, list(range(N_CORES))).results
    out16 = np.concatenate([res[c]["out"] for c in range(N_CORES)], axis=0)
    return out16.astype(np.float32)


# revision 18
# speedup vs baseline: 1.2039x; 1.0370x over previous
"""Trainium2 Bass kernel for nn_Mix_82360292868539.

reference math:
    inner = x @ y.T                                   # [8192, 8192] fp32
    pdist = sx[:,None] + sy[None,:] - 2*inner
    sigma = median(pdist) / (2*log(8193))
    kxy   = exp(-pdist/sigma/2) + 0.1*(inner + 0)**2

Design (vs the 185us baseline, which was ACT-bound: Square+Exp both on the
scalar engine = 16M 1x-elems/core = ~128us busy):

  * sigma from a host-side subsample median (x[::16] vs y[::16], 262144
    samples, rel SE ~4e-4 on the median). The exp term is only ~1.4e-4 of
    the output L2 norm, so sigma precision is irrelevant at the 2e-2 gate.
    This deletes the entire device pass 1 (~35us).
  * fp16 output: halves the dominant out-DMA (32MB -> 16MB per core).
  * forked PSUM chains, one [128,1024] tile pair per group:
      MM_P = C1*x.yh                     (pure inner; ACT squares it)
      MM_E = C1*x~.yh - C1*(sx_i+sy_j)/2 (= -C1*pdist/2; the full aug rides
             in two sacrificial K-rows: lhsT [uhi; ulo[:62]; ax; 1],
             rhs [yh; yh[:62]; 1; cy], so x~ drops x_lo features 62-63,
             err ~4e-4 on the exp arg)
    No square->aug->exp WAR chain; each PSUM bank has one writer then one
    reader, so 2 bufs per pool pipeline cleanly.
  * exp WITHOUT the scalar engine: Schraudolph bit-trick on DVE. One
    tensor_scalar from PSUM computes i = W_E*(A16/(C1*sigma)) + B16 with
    uint16 output; the uint16 bit pattern IS fp16(exp(z)) (exponent-field
    linear interpolation, max rel err ~3%, invisible at E's 1.4e-4 L2
    share). Negative i (exp underflow, z < -10.4) saturates to 0 on the
    unsigned convert = exactly the clamp needed. The add then reads the
    tile bitcast as fp16.
  * adds in fp16 (DVE 2x_1P mode), a slice of groups routed to GpSimd, and
    a slice of exps routed to ACT (exact Exp), to balance the three
    engines. add_pair=True merges each jg pair's square/exp outputs into
    double-width [128,2048] tiles: one add + one 4KB-line out-DMA per pair
    (measured ~23% faster than per-group adds). sbuf_bufs=6 gives the
    wider staging tiles more pipeline depth. Knobs: exp_act_mod, gp_mod,
    add_pair, mm_pair, sbuf_bufs.

Error budget: P misses x.ylo (~2.3e-3 L2, dominant), fp16 out ~2.4e-4,
Schraudolph ~4e-6, sigma ~1e-6. Total ~2.4e-3 vs the 2e-2 gate.
"""

import math
import numpy as np

import jax
from jax.sharding import Mesh, PartitionSpec, NamedSharding
from jax.experimental.shard_map import shard_map

import bass_rust
import ml_dtypes
import concourse.bass as bass
import concourse.mybir as mybir
from concourse.tile import TileContext

BF16 = ml_dtypes.bfloat16

N, M, D = 8192, 8192, 64
R_POLY = 0.1
N_CORES = 8
ROWS = N // N_CORES          # 1024 rows per core
C1 = math.sqrt(R_POLY)       # sqrt(0.1) folded into x side of the matmuls

F_TILE = 1024                # output-tile free dim (2 PSUM banks)
RB = ROWS // 128             # row blocks per core (8)
CT = M // F_TILE             # column tiles (8)

# fp16 Schraudolph constants: bitcast_u16(round(A16*z + B16)) ~= fp16(exp(z))
A16 = 1024.0 / math.log(2.0)          # 2^10 * log2(e)
B16 = (15.0 - 0.0434609) * 1024.0 + 0.5   # +0.5: correct under truncation


def _split_multiwait_ctrl(nc, maxw=1):
    """This container's walrus build only accepts one sem-wait command per
    instruction. Split any multi-wait instruction into a chain of
    single-wait NoOps (same engine, program order preserved) followed by
    the original instruction carrying the final wait."""
    for f in nc.m.functions:
        for bb in f.blocks:
            new = []
            for inst in bb.instructions:
                si = inst.sync_info
                ws = list(si.on_wait) if si and si.on_wait else []
                if len(ws) > maxw and inst.engine is not None:
                    for i, w in enumerate(ws[:-maxw]):
                        d = mybir.InstNoOp(name=f"{inst.name}-sw{i}", ins=[], outs=[])
                        d.engine = inst.engine
                        d.sync_info = bass_rust.SyncInfo(on_wait=[w], on_update=[])
                        new.append(d)
                    si.on_wait = ws[-maxw:]
                new.append(inst)
            bb.instructions = new


def build_pass2(exp_act_mod=4, gp_mod=0, sbuf_bufs=6, repeat=1, timing=False,
                no_dma=False, pair_dma=False, add_pair=True, mm_pair=False):
    """exp_act_mod: every exp_act_mod-th group's exp runs exactly on ACT
    instead of the DVE bit-trick (0 = all DVE). gp_mod: every gp_mod-th
    group's add runs on GpSimd instead of DVE (0 = all DVE)."""
    nc = bass.Bass("TRN2", target_bir_lowering=False, num_devices=N_CORES)
    u2T = nc.dram_tensor("u2T", [128, ROWS], mybir.dt.bfloat16, kind="ExternalInput")
    uET = nc.dram_tensor("uET", [128, ROWS], mybir.dt.bfloat16, kind="ExternalInput")
    yPT = nc.dram_tensor("yPT", [128, M], mybir.dt.bfloat16, kind="ExternalInput")
    yET = nc.dram_tensor("yET", [128, M], mybir.dt.bfloat16, kind="ExternalInput")
    sAin = nc.dram_tensor("sA", [128, 1], mybir.dt.float32, kind="ExternalInput")
    c2in = nc.dram_tensor("c2", [128, 1], mybir.dt.float32, kind="ExternalInput")
    if timing:
        out = nc.dram_tensor("scratch", [ROWS, M], mybir.dt.float16, kind="Internal")
        tok = nc.dram_tensor("tok", [128, 8], mybir.dt.float16, kind="ExternalOutput")
    else:
        out = nc.dram_tensor("out", [ROWS, M], mybir.dt.float16, kind="ExternalOutput")

    with TileContext(nc) as tc:
        with tc.tile_pool(name="w", bufs=1) as wpool, \
             tc.tile_pool(name="psP", bufs=2, space="PSUM") as ppsum, \
             tc.tile_pool(name="psE", bufs=2, space="PSUM") as epsum, \
             tc.tile_pool(name="pb", bufs=sbuf_bufs) as ppool, \
             tc.tile_pool(name="eb", bufs=sbuf_bufs) as epool, \
             tc.tile_pool(name="ob", bufs=sbuf_bufs) as opool:
            u2 = wpool.tile([128, ROWS], mybir.dt.bfloat16)
            nc.sync.dma_start(out=u2, in_=u2T[:, :])
            uE = wpool.tile([128, ROWS], mybir.dt.bfloat16)
            nc.sync.dma_start(out=uE, in_=uET[:, :])
            sA = wpool.tile([128, 1], mybir.dt.float32)
            nc.sync.dma_start(out=sA, in_=sAin[:, :])
            c2 = wpool.tile([128, 1], mybir.dt.float32)
            nc.sync.dma_start(out=c2, in_=c2in[:, :])
            yP_ch, yE_ch = [], []
            for j in range(CT):
                csl = slice(j * F_TILE, (j + 1) * F_TILE)
                tp = wpool.tile([128, F_TILE], mybir.dt.bfloat16, tag=f"yp{j}")
                nc.sync.dma_start(out=tp, in_=yPT[:, csl])
                yP_ch.append(tp)
                te = wpool.tile([128, F_TILE], mybir.dt.bfloat16, tag=f"ye{j}")
                nc.sync.dma_start(out=te, in_=yET[:, csl])
                yE_ch.append(te)

            g = 0
            pending = None  # (pt, et_fp16_view, rsl, jg) awaiting add + dma
            pair_state = {}  # jg parity -> shared double-width ot tile

            def flush(pend, g_):
                pt_, etv_, rsl_, jg_ = pend
                if add_pair:
                    dsl = slice((jg_ - 1) * F_TILE, (jg_ + 1) * F_TILE)
                    ot = opool.tile([128, 2 * F_TILE], mybir.dt.float16,
                                    tag="otw")
                    if gp_mod and (g_ // 2) % gp_mod == 0:
                        nc.gpsimd.tensor_tensor(ot, pt_, etv_,
                                                mybir.AluOpType.add)
                    else:
                        nc.vector.tensor_tensor(ot, pt_, etv_,
                                                mybir.AluOpType.add)
                    if not no_dma:
                        nc.sync.dma_start(out=out[rsl_, dsl], in_=ot)
                    return ot
                osl_ = slice(jg_ * F_TILE, (jg_ + 1) * F_TILE)
                if pair_dma:
                    # two adjacent column groups share one [128, 2*F_TILE]
                    # tile so the out-DMA moves 4KB per partition line
                    if jg_ % 2 == 0:
                        ot_new = opool.tile(
                            [128, 2 * F_TILE], mybir.dt.float16, tag="otp")
                        pair_state["ot"] = ot_new
                    ot2 = pair_state["ot"]
                    half = ot2[:, (jg_ % 2) * F_TILE:(jg_ % 2 + 1) * F_TILE]
                    if gp_mod and g_ % gp_mod == 0:
                        nc.gpsimd.tensor_tensor(half, pt_, etv_,
                                                mybir.AluOpType.add)
                    else:
                        nc.vector.tensor_tensor(half, pt_, etv_,
                                                mybir.AluOpType.add)
                    if jg_ % 2 == 1 and not no_dma:
                        dsl = slice((jg_ - 1) * F_TILE, (jg_ + 1) * F_TILE)
                        nc.sync.dma_start(out=out[rsl_, dsl], in_=ot2)
                    return ot2
                ot = opool.tile([128, F_TILE], mybir.dt.float16)
                if gp_mod and g_ % gp_mod == 0:
                    nc.gpsimd.tensor_tensor(ot, pt_, etv_, mybir.AluOpType.add)
                else:
                    nc.vector.tensor_tensor(ot, pt_, etv_, mybir.AluOpType.add)
                if not no_dma:
                    nc.sync.dma_start(out=out[rsl_, osl_], in_=ot)
                return ot

            last_ot = None
            pf = {}  # shared double-width pt/et tiles for add_pair mode
            for _rep in range(repeat):
              for rb in range(RB):
                rsl = slice(rb * 128, (rb + 1) * 128)
                mmq = {}  # jg -> (psP, psE) when mm_pair pre-issues them
                for jg in range(CT):
                    osl = slice(jg * F_TILE, (jg + 1) * F_TILE)
                    if mm_pair and jg % 2 == 0:
                        # burst both groups' MM_P under one stationary load,
                        # then both MM_E, halving lhsT switches on the PE
                        ps = {}
                        for j2 in (jg, jg + 1):
                            pj = ppsum.tile([128, F_TILE], mybir.dt.float32,
                                            tag="psPq")
                            for h in range(F_TILE // 512):
                                hsl = slice(h * 512, (h + 1) * 512)
                                nc.tensor.matmul(pj[:, hsl], lhsT=u2[:, rsl],
                                                 rhs=yP_ch[j2][:, hsl],
                                                 start=True, stop=True)
                            ps[j2] = pj
                        for j2 in (jg, jg + 1):
                            ej = epsum.tile([128, F_TILE], mybir.dt.float32,
                                            tag="psEq")
                            for h in range(F_TILE // 512):
                                hsl = slice(h * 512, (h + 1) * 512)
                                nc.tensor.matmul(ej[:, hsl], lhsT=uE[:, rsl],
                                                 rhs=yE_ch[j2][:, hsl],
                                                 start=True, stop=True)
                            mmq[j2] = (ps[j2], ej)
                    if mm_pair:
                        psP, psE = mmq[jg]
                    else:
                        psP = ppsum.tile([128, F_TILE], mybir.dt.float32)
                        for h in range(F_TILE // 512):
                            hsl = slice(h * 512, (h + 1) * 512)
                            nc.tensor.matmul(psP[:, hsl], lhsT=u2[:, rsl],
                                             rhs=yP_ch[jg][:, hsl],
                                             start=True, stop=True)
                        psE = epsum.tile([128, F_TILE], mybir.dt.float32)
                        for h in range(F_TILE // 512):
                            hsl = slice(h * 512, (h + 1) * 512)
                            nc.tensor.matmul(psE[:, hsl], lhsT=uE[:, rsl],
                                             rhs=yE_ch[jg][:, hsl],
                                             start=True, stop=True)
                    if add_pair:
                        # square/exp of a jg pair write halves of [128, 2*F]
                        # tiles; one add + one DMA per pair
                        if jg % 2 == 0:
                            pt2 = ppool.tile([128, 2 * F_TILE],
                                             mybir.dt.float16, tag="ptp")
                            et2 = epool.tile([128, 2 * F_TILE],
                                             mybir.dt.uint16, tag="etp")
                            pf = {"pt2": pt2, "et2": et2}
                        hs = slice((jg % 2) * F_TILE, (jg % 2 + 1) * F_TILE)
                        nc.scalar.activation(
                            pf["pt2"][:, hs], psP,
                            mybir.ActivationFunctionType.Square)
                        if exp_act_mod and g % exp_act_mod == 0:
                            nc.scalar.activation(
                                pf["et2"][:, hs].bitcast(mybir.dt.float16),
                                psE, mybir.ActivationFunctionType.Exp,
                                scale=c2[:, :])
                        else:
                            nc.vector.tensor_scalar(
                                pf["et2"][:, hs], psE, sA[:, :], B16,
                                mybir.AluOpType.mult, mybir.AluOpType.add)
                        if jg % 2 == 1:
                            if pending is not None:
                                last_ot = flush(pending, g - 1)
                            pending = (pf["pt2"],
                                       pf["et2"][:, :].bitcast(
                                           mybir.dt.float16),
                                       rsl, jg)
                        g += 1
                        continue
                    # P = W_P^2 = 0.1*(x.yh)^2   (ACT; DVE can't square PSUM)
                    pt = ppool.tile([128, F_TILE], mybir.dt.float16)
                    nc.scalar.activation(
                        pt, psP, mybir.ActivationFunctionType.Square)
                    # E = exp(W_E/(C1*sigma))
                    if exp_act_mod and g % exp_act_mod == 0:
                        et = epool.tile([128, F_TILE], mybir.dt.float16)
                        nc.scalar.activation(
                            et, psE, mybir.ActivationFunctionType.Exp,
                            scale=c2[:, :])
                        etv = et[:, :]
                    else:
                        et = epool.tile([128, F_TILE], mybir.dt.uint16)
                        nc.vector.tensor_scalar(
                            et, psE, sA[:, :], B16,
                            mybir.AluOpType.mult, mybir.AluOpType.add)
                        etv = et[:, :].bitcast(mybir.dt.float16)
                    if pending is not None:
                        last_ot = flush(pending, g - 1)
                    pending = (pt, etv, rsl, jg)
                    g += 1
              if pending is not None:
                  last_ot = flush(pending, g - 1)
                  pending = None
              if timing:
                  nc.sync.dma_start(out=tok[:, :], in_=last_ot[:, 0:8])
    _split_multiwait_ctrl(nc)
    return nc


class BassRunner:
    """Persistent PJRT executor for a Bass program."""

    def __init__(self, nc, n_cores):
        from concourse.bass2jax import (
            _bass_exec_p, install_neuronx_cc_hook, partition_id_tensor)
        install_neuronx_cc_hook()
        self.nc = nc
        self.n_cores = n_cores
        partition_name = (
            nc.partition_id_tensor.name if nc.partition_id_tensor else None)

        in_names, out_names, out_avals = [], [], []
        for alloc in nc.m.functions[0].allocations:
            if not isinstance(alloc, mybir.MemoryLocationSet):
                continue
            name = alloc.memorylocations[0].name
            if alloc.kind == "ExternalInput":
                if name != partition_name:
                    in_names.append(name)
            elif alloc.kind == "ExternalOutput":
                out_names.append(name)
                out_avals.append(jax.core.ShapedArray(
                    tuple(alloc.tensor_shape), mybir.dt.np(alloc.dtype)))
        self.in_names = in_names
        self.out_names = out_names
        self.out_avals = out_avals
        all_in_names = in_names + out_names
        if partition_name is not None:
            all_in_names.append(partition_name)

        def _body(*args):
            operands = list(args)
            if partition_name is not None:
                operands.append(partition_id_tensor())
            return tuple(_bass_exec_p.bind(
                *operands,
                out_avals=tuple(out_avals),
                in_names=tuple(all_in_names),
                out_names=tuple(out_names),
                lowering_input_output_aliases=(),
                sim_require_finite=True,
                sim_require_nnan=True,
                nc=nc,
            ))

        devices = jax.devices()[:n_cores]
        self.mesh = Mesh(np.asarray(devices), ("core",))
        self.sharding = NamedSharding(self.mesh, PartitionSpec("core"))
        n_total = len(in_names) + len(out_names)
        self.jitted = jax.jit(
            shard_map(_body, mesh=self.mesh,
                      in_specs=(PartitionSpec("core"),) * n_total,
                      out_specs=(PartitionSpec("core"),) * len(out_names),
                      check_rep=False),
            keep_unused=True,
        )
        self._zero_dev = None

    def stage_inputs(self, in_maps):
        return [
            jax.device_put(
                np.concatenate([np.asarray(m[name]) for m in in_maps], axis=0),
                self.sharding)
            for name in self.in_names
        ]

    def zero_carriers(self):
        if self._zero_dev is None:
            self._zero_dev = [
                jax.device_put(
                    np.zeros((self.n_cores * av.shape[0], *av.shape[1:]),
                             av.dtype), self.sharding)
                for av in self.out_avals
            ]
        return self._zero_dev

    def execute(self, dev_inputs):
        outs = self.jitted(*dev_inputs, *self.zero_carriers())
        for o in outs:
            o.block_until_ready()
        return outs

    def run(self, in_maps):
        outs = self.execute(self.stage_inputs(in_maps))
        res = []
        for c in range(self.n_cores):
            d = {}
            for i, name in enumerate(self.out_names):
                av = self.out_avals[i]
                d[name] = np.asarray(outs[i]).reshape(
                    self.n_cores, *av.shape)[c]
            res.append(d)
        return res


def _bf16_split(a):
    hi = a.astype(BF16)
    lo = (a - hi.astype(np.float32)).astype(BF16)
    return hi, lo


def _sigma(x, y):
    """Median of pdist over a deterministic 512x512 subsample (262144
    entries; rel SE ~4e-4 on the median, invisible at the 2e-2 gate)."""
    xs, ys = x[::16], y[::16]
    inner = xs @ ys.T
    pd = ((xs * xs).sum(1)[:, None] + (ys * ys).sum(1)[None, :]
          - 2.0 * inner)
    return float(np.median(pd)) / (2.0 * math.log(np.float32(N + 1)))


def make_in_maps(x, y, sigma):
    """Host-side prep of all per-core device inputs."""
    sx = (x * x).sum(1)
    sy = (y * y).sum(1)
    u = (C1 * x).astype(np.float32)
    u_hi, u_lo = _bf16_split(u)
    y_hi = y.astype(BF16)
    ax = (-0.5 * C1 * sx).astype(np.float32).astype(BF16)[:, None]
    cy = (-0.5 * C1 * sy).astype(np.float32).astype(BF16)[:, None]
    ones_n = np.ones((N, 1), dtype=BF16)

    # MM_P stationary: [u_hi; u_lo].T  (K=128)
    u2T_full = np.ascontiguousarray(np.concatenate([u_hi, u_lo], axis=1).T)
    # MM_E stationary: [u_hi; u_lo[:, :62]; ax; ones].T
    uET_full = np.ascontiguousarray(
        np.concatenate([u_hi, u_lo[:, :62], ax, ones_n], axis=1).T)
    # moving operands: [yh; yh] and [yh; yh[:, :62]; ones; cy]
    yPT = np.ascontiguousarray(np.concatenate([y_hi, y_hi], axis=1).T)
    yET = np.ascontiguousarray(
        np.concatenate([y_hi, y_hi[:, :62], ones_n, cy], axis=1).T)
    sA = np.full((128, 1), A16 / (C1 * sigma), dtype=np.float32)
    c2 = np.full((128, 1), 1.0 / (C1 * sigma), dtype=np.float32)

    in_maps = []
    for c in range(N_CORES):
        rsl = slice(c * ROWS, (c + 1) * ROWS)
        in_maps.append({
            "u2T": np.ascontiguousarray(u2T_full[:, rsl]),
            "uET": np.ascontiguousarray(uET_full[:, rsl]),
            "yPT": yPT,
            "yET": yET,
            "sA": sA,
            "c2": c2,
        })
    return in_maps


_CACHE = {}


def _runner():
    if "r2" not in _CACHE:
        _CACHE["r2"] = BassRunner(build_pass2(), N_CORES)
    return _CACHE["r2"]


def kernel(x: np.ndarray, y: np.ndarray) -> np.ndarray:
    x = np.ascontiguousarray(np.asarray(x, dtype=np.float32))
    y = np.ascontiguousarray(np.asarray(y, dtype=np.float32))
    assert x.shape == (N, D) and y.shape == (M, D)

    sigma = _sigma(x, y)
    in_maps = make_in_maps(x, y, sigma)
    try:
        res = _runner().run(in_maps)
    except Exception:
        from concourse.bass_utils import run_bass_kernel_spmd
        res = run_bass_kernel_spmd(
            build_pass2(), in_maps, list(range(N_CORES))).results
    out16 = np.concatenate([res[c]["out"] for c in range(N_CORES)], axis=0)
    return out16.astype(np.float32)
